# Initial kernel scaffold
#
"""DiffLinearAttentionWeights Trainium2 kernel.

Math (per b, h):
    aw_i = (q @ Wq_i) @ (k @ Wk_i)^T  = q @ M_i @ k^T,   M_i = Wq_i @ Wk_i^T
    masked with tril(k=1), row-normalized; out = aw_1/den_1 - lam * aw_2/den_2.

Key factorizations used on-device:
  * U_i = q @ M_i  (so aw_i = U_i @ k^T, contraction over D=64).
  * den_i[t] = sum_{s<=t+1} aw_i[t,s] = U_i[t] . P[t], where P[t] is the
    (shifted) prefix-sum of k rows -> computed with one DVE scan, no O(T^2) work.
  * Normalization + lambda-combination folded into the matmul: scale the
    stacked U^T columns by [1/den_1 ; -lam/den_2] and do ONE K=128 matmul
        out_tile = [U1s; U2s]^T @ [k^T; k^T]
    per 128x512 output tile (only tiles under the causal profile).
  * Output rows above the causal profile are never written; the PJRT output
    buffers are donated zero-filled arrays, so skipped regions stay zero.

Sharding: BH = 64 (b,h) pairs, 8 per core, SPMD on 8 NeuronCores.
"""

import math
import sys

sys.path.insert(0, "/opt/trn_rl_repo")

import numpy as np

B, H, T, D = 4, 16, 1024, 64
NCORES = 8
BH = B * H
JPC = BH // NCORES          # bh pairs per core
NT = T // 128               # t-chunks of 128 rows
DEPTH = 12
LAMBDA_INIT = 0.8 - 0.6 * math.exp(-0.3 * DEPTH)

# live width of output row-block i: causal tril(k=1) keeps cols 0..128*(i+1)+1
def _live_width(i):
    return min(128 * (i + 1) + 1, T)


_BUILD_CACHE = {}


def _build_module(n_bh=JPC, aw_f32r=False):
    """Trace + compile the per-core Bass module (cached)."""
    key = (n_bh, aw_f32r)
    if key in _BUILD_CACHE:
        return _BUILD_CACHE[key]

    import concourse.bass as bass
    import concourse.mybir as mybir
    import concourse.bacc as bacc
    import concourse.tile as tile
    from concourse import masks

    fp32 = mybir.dt.float32
    f32r = mybir.dt.float32r
    P = 128

    nc = bacc.Bacc("TRN2", target_bir_lowering=False, debug=False,
                   enable_asserts=False)

    q_d = nc.dram_tensor("q", [n_bh, T, D], fp32, kind="ExternalInput")
    k_d = nc.dram_tensor("k", [n_bh, T, D], fp32, kind="ExternalInput")
    # m_stack[j] = [M1 | M2]  (64 x 128)
    ms_d = nc.dram_tensor("ms", [n_bh, D, 2 * D], fp32, kind="ExternalInput")
    # ones128: cols 0..63 = 1 on rows 0..63; cols 64..127 = -1/lam on rows 64..127
    on_d = nc.dram_tensor("on", [P, P], fp32, kind="ExternalInput")
    out_d = nc.dram_tensor("out", [n_bh, T, T], fp32, kind="ExternalOutput")

    with tile.TileContext(nc) as tc:
        with tc.tile_pool(name="const", bufs=1) as cpool, \
             tc.tile_pool(name="stage", bufs=3) as stage, \
             tc.tile_pool(name="big", bufs=2) as big, \
             tc.tile_pool(name="outp", bufs=3) as outp, \
             tc.tile_pool(name="trp", bufs=2, space=bass.MemorySpace.PSUM) as trp, \
             tc.tile_pool(name="usp", bufs=1, space=bass.MemorySpace.PSUM) as usp, \
             tc.tile_pool(name="denp", bufs=1, space=bass.MemorySpace.PSUM) as denp, \
             tc.tile_pool(name="owp", bufs=2, space=bass.MemorySpace.PSUM) as owp:

            # ---- constants ----
            ident = cpool.tile([P, P], fp32)
            masks.make_identity(nc, ident[:])
            # tril(k=1) multiplicative mask for the diagonal 128x129 strip
            mdc = cpool.tile([P, 132], fp32)
            nc.gpsimd.memset(mdc[:], 1.0)
            nc.gpsimd.affine_select(
                out=mdc[:], in_=mdc[:], compare_op=mybir.AluOpType.is_ge,
                fill=0.0, base=1, pattern=[[-1, 132]], channel_multiplier=1)
            on_sb = cpool.tile([P, P], fp32)
            nc.sync.dma_start(on_sb[:], on_d[:])
            ms_sb = cpool.tile([D, n_bh, 2 * D], fp32)
            nc.sync.dma_start(ms_sb[:], ms_d.rearrange("j d m -> d j m"))

            for j in range(n_bh):
                # ---- load q, k (t-chunk partition layout) ----
                q_sb = stage.tile([P, NT, D], fp32, tag="q_sb")
                k_sb = stage.tile([P, NT, D], fp32, tag="k_sb")
                nc.sync.dma_start(q_sb[:], q_d[j].rearrange("(c p) d -> p c d", p=P))
                nc.sync.dma_start(k_sb[:], k_d[j].rearrange("(c p) d -> p c d", p=P))

                # ---- transposes: qT [64, 1024], kT2 [128, 1024] (dup halves) ----
                kt_dt = f32r if aw_f32r else fp32
                qT2 = big.tile([D, T], fp32, tag="qT2")
                kT2 = big.tile([P, T], kt_dt, tag="kT2")
                for src, dst in ((q_sb, qT2), (k_sb, kT2)):
                    for g in range(2):       # groups of 4 chunks
                        tp = trp.tile([D, 512], fp32, tag="tr")
                        for cc in range(4):
                            c = g * 4 + cc
                            nc.tensor.transpose(tp[:, 128 * cc:128 * (cc + 1)],
                                                src[:, c, :], ident[:])
                        nc.scalar.copy(dst[0:D, 512 * g:512 * (g + 1)], tp[:])
                # duplicate kT into partitions 64..127 (DMA moves across partitions)
                nc.sync.dma_start(kT2[D:P, :], kT2[0:D, :])

                # ---- Ustack = [U1^T ; U2^T]  [128, 1024] ----
                ust = big.tile([P, T], fp32, tag="ust")
                for g in range(2):
                    up = usp.tile([P, 512], fp32, tag="us")
                    nc.tensor.matmul(up[:], ms_sb[:, j, :],
                                     qT2[0:D, 512 * g:512 * (g + 1)])
                    nc.scalar.copy(ust[:, 512 * g:512 * (g + 1)], up[:])

                # ---- prefix sums C2 (cumsum of k over t, stacked) ----
                c2 = big.tile([P, T], fp32, tag="c2")
                nc.vector.tensor_tensor_scan(c2[:], kT2[:], kT2[:], 0.0,
                                             mybir.AluOpType.add,
                                             mybir.AluOpType.bypass)

                # ---- W = Ustack * shifted(C2);  den = ones128^T @ W ----
                w_sb = big.tile([P, T], fp32, tag="w")
                nc.vector.tensor_mul(w_sb[:, 0:T - 1], ust[:, 0:T - 1], c2[:, 1:T])
                nc.vector.tensor_mul(w_sb[:, T - 1:T], ust[:, T - 1:T], c2[:, T - 1:T])

                rden = big.tile([P, T], fp32, tag="rden")
                for g in range(2):
                    dp = denp.tile([P, 512], fp32, tag="den")
                    nc.tensor.matmul(dp[:], on_sb[:], w_sb[:, 512 * g:512 * (g + 1)])
                    # reciprocal: rows 0..63 = 1/den1, rows 64..127 = -lam/den2
                    nc.vector.reciprocal(rden[:, 512 * g:512 * (g + 1)], dp[:])

                # ---- V = Ustack * rden (normalization + lambda folded in) ----
                v_sb = big.tile([P, T], kt_dt, tag="v")
                nc.vector.tensor_mul(v_sb[:], ust[:], rden[:])
                v_mm = v_sb[:]
                kt_mm = kT2[:]

                # ---- output tiles ----
                for i in range(NT):
                    wl = _live_width(i)
                    ops = owp.tile([P, 1024], fp32, tag="ow")
                    n0 = min(wl, 512)
                    nc.tensor.matmul(ops[:, 0:n0],
                                     v_mm[:, 128 * i:128 * (i + 1)],
                                     kt_mm[:, 0:n0])
                    if wl > 512:
                        nc.tensor.matmul(ops[:, 512:wl],
                                         v_mm[:, 128 * i:128 * (i + 1)],
                                         kt_mm[:, 512:wl])

                    osb = outp.tile([P, 1032], fp32, tag="osb")
                    # masked diagonal strip (includes the +1 superdiagonal col)
                    mw = wl - 128 * i
                    nc.vector.tensor_mul(osb[:, 128 * i:wl],
                                         ops[:, 128 * i:wl], mdc[:, 0:mw])
                    # full-keep columns
                    if i > 0:
                        if i <= 2:
                            nc.vector.tensor_copy(osb[:, 0:128 * i],
                                                  ops[:, 0:128 * i])
                        else:
                            nc.scalar.copy(osb[:, 0:128 * i],
                                           ops[:, 0:128 * i])
                    nc.sync.dma_start(
                        out_d[j, 128 * i:128 * (i + 1), 0:wl], osb[:, 0:wl])

    nc.compile()
    _BUILD_CACHE[key] = nc
    return nc


def _host_prep(W1_q, W1_k, W2_q, W2_k, lambda_q1, lambda_k1, lambda_q2,
               lambda_k2):
    lam1 = np.exp(np.asarray(lambda_q1, np.float64).dot(
        np.asarray(lambda_k1, np.float64)))
    lam2 = np.exp(np.asarray(lambda_q2, np.float64).dot(
        np.asarray(lambda_k2, np.float64)))
    lam = np.float32(np.float32(lam1) - np.float32(lam2) + np.float32(LAMBDA_INIT))
    M1 = np.einsum("hde,hfe->hdf", W1_q.astype(np.float32),
                   W1_k.astype(np.float32)).astype(np.float32)
    M2 = np.einsum("hde,hfe->hdf", W2_q.astype(np.float32),
                   W2_k.astype(np.float32)).astype(np.float32)
    m_stack = np.concatenate([M1, M2], axis=2)          # [H, 64, 128]
    ones = np.zeros((128, 128), np.float32)
    ones[0:64, 0:64] = 1.0
    ones[64:128, 64:128] = np.float32(-1.0) / lam
    return m_stack, ones


def _make_in_maps(query_states, key_states, W1_q, W1_k, W2_q, W2_k,
                  lambda_q1, lambda_k1, lambda_q2, lambda_k2):
    q = np.ascontiguousarray(np.asarray(query_states, np.float32).reshape(BH, T, D))
    k = np.ascontiguousarray(np.asarray(key_states, np.float32).reshape(BH, T, D))
    m_stack, ones = _host_prep(W1_q, W1_k, W2_q, W2_k,
                               lambda_q1, lambda_k1, lambda_q2, lambda_k2)
    in_maps = []
    for c in range(NCORES):
        sl = slice(c * JPC, (c + 1) * JPC)
        hs = [bh % H for bh in range(c * JPC, (c + 1) * JPC)]
        in_maps.append({
            "q": np.ascontiguousarray(q[sl]),
            "k": np.ascontiguousarray(k[sl]),
            "ms": np.ascontiguousarray(m_stack[hs]),
            "on": ones,
        })
    return in_maps


def kernel(query_states, key_states, W1_q, W1_k, W2_q, W2_k,
           lambda_q1, lambda_k1, lambda_q2, lambda_k2):
    from concourse.bass_utils import run_bass_kernel_spmd

    in_maps = _make_in_maps(query_states, key_states, W1_q, W1_k, W2_q, W2_k,
                            lambda_q1, lambda_k1, lambda_q2, lambda_k2)
    nc = _build_module()
    res = run_bass_kernel_spmd(nc, in_maps, core_ids=list(range(NCORES)),
                               trace=False)
    out = np.empty((BH, T, T), np.float32)
    for c in range(NCORES):
        out[c * JPC:(c + 1) * JPC] = res.results[c]["out"]
    return out.reshape(B, H, T, T)



# revision 26
# speedup vs baseline: 3.4200x; 3.4200x over previous
"""DiffLinearAttentionWeights Trainium2 kernel.

Math (per b, h):
    aw_i = (q @ Wq_i) @ (k @ Wk_i)^T  = q @ M_i @ k^T,   M_i = Wq_i @ Wk_i^T
    masked with tril(k=1), row-normalized; out = aw_1/den_1 - lam * aw_2/den_2.

Key factorizations used on-device:
  * U_i = q @ M_i  (so aw_i = U_i @ k^T, contraction over D=64).
  * den_i[t] = sum_{s<=t+1} aw_i[t,s] = U_i[t] . P[t+1], where P is the
    inclusive prefix-sum of k rows -> one DVE scan, no O(T^2) work.
  * Normalization + lambda-combination folded into the matmul: scale the
    stacked U^T columns by [1/den_1 ; -lam/den_2] and do ONE K=128 matmul
        out_tile = [U1s; U2s]^T @ [k^T; k^T]
    per 128-row output tile (only the columns under the causal profile).
  * Output rows above the causal profile are never written; the PJRT output
    buffers are donated zero-filled arrays, so skipped regions stay zero.

Performance structure (per bh pair):
  * q and k are host-swizzled to [p, chunk, d] so input DMAs move 2KB
    contiguous per partition (full-rate descriptors).
  * One [128,128] PE transpose per t-chunk of the interleaved [q|k] pair
    produces qT on partitions 0:64 and kT on 64:128 in one pass.  After the
    U matmul consumes qT, a single SBUF->SBUF DMA duplicates kT over it,
    giving the stacked [k^T; k^T] operand in place.
  * The big output matmul runs in float32r (TF32, 1 PE cycle/row vs 4 for
    fp32).  The U/den/normalization path stays full fp32: the row sums
    cancel catastrophically, so den must match the fp32 reference closely.
  * Output tiles are written as bf16 (halves output DMA bytes; rel tol is
    2e-2, bf16 rounding adds <=4e-3 per element); host converts to fp32.
  * PSUM->SBUF conversions are spread across Activation and Pool engines.

Sharding: BH = 64 (b,h) pairs, 8 per core, SPMD on 8 NeuronCores.
"""

import math
import sys

sys.path.insert(0, "/opt/trn_rl_repo")

import numpy as np

B, H, T, D = 4, 16, 1024, 64
NCORES = 8
BH = B * H
JPC = BH // NCORES          # bh pairs per core
NT = T // 128               # t-chunks of 128 rows
DEPTH = 12
LAMBDA_INIT = 0.8 - 0.6 * math.exp(-0.3 * DEPTH)

# live width of output row-block i: causal tril(k=1) keeps cols 0..128*(i+1)+1
def _live_width(i):
    return min(128 * (i + 1) + 1, T)


_BUILD_CACHE = {}


def _build_module(n_bh=JPC, repeat=1, use_f32r=True, out_bf16=True):
    """Trace + compile the per-core Bass module (cached).

    repeat > 1 wraps the whole per-core body in a hardware loop running the
    identical (idempotent) computation `repeat` times -- used only by the
    timing harness to amortize dispatch overhead.
    """
    key = (n_bh, repeat, use_f32r, out_bf16)
    if key in _BUILD_CACHE:
        return _BUILD_CACHE[key]

    import concourse.bass as bass
    import concourse.mybir as mybir
    import concourse.bacc as bacc
    import concourse.tile as tile
    from concourse import masks

    fp32 = mybir.dt.float32
    f32r = mybir.dt.float32r if use_f32r else mybir.dt.float32
    bf16 = mybir.dt.bfloat16 if out_bf16 else mybir.dt.float32
    P = 128

    nc = bacc.Bacc("TRN2", target_bir_lowering=False, debug=False,
                   enable_asserts=False)

    # host-interleaved: element [j, p, c, 0, d] = q[j, c*128+p, d],
    #                   element [j, p, c, 1, d] = k[j, c*128+p, d]
    qk_d = nc.dram_tensor("qk", [n_bh, P, NT, 2 * D], fp32,
                          kind="ExternalInput")
    # m_stack[j] = [M1 | M2]  (64 x 128)
    ms_d = nc.dram_tensor("ms", [n_bh, D, 2 * D], fp32, kind="ExternalInput")
    # block-diagonal ones (rows 0:64 x cols 0:64 and 64:128 x 64:128) --
    # exact in TF32, so the den matmul can run as 2-pass f32r
    on_d = nc.dram_tensor("on", [P, P], fp32, kind="ExternalInput")
    # per-partition scale column: rows 0:64 = 1.0, rows 64:128 = -lambda
    sc_d = nc.dram_tensor("sc", [P, 1], fp32, kind="ExternalInput")
    out_d = nc.dram_tensor("out", [n_bh, T, T], bf16, kind="ExternalOutput")

    with tile.TileContext(nc) as tc:
        with tc.tile_pool(name="const", bufs=1) as cpool, \
             tc.tile_pool(name="stage", bufs=3) as stage, \
             tc.tile_pool(name="big", bufs=3) as big, \
             tc.tile_pool(name="outp", bufs=4) as outp, \
             tc.tile_pool(name="trp", bufs=2, space=bass.MemorySpace.PSUM) as trp, \
             tc.tile_pool(name="usp", bufs=1, space=bass.MemorySpace.PSUM) as usp, \
             tc.tile_pool(name="denp", bufs=1, space=bass.MemorySpace.PSUM) as denp, \
             tc.tile_pool(name="owp", bufs=2, space=bass.MemorySpace.PSUM) as owp:

            # ---- constants ----
            ident = cpool.tile([P, P], fp32)
            masks.make_identity(nc, ident[:])
            # combined causal mask: cols 0..1023 are all-ones (full-keep body),
            # cols 1024..1155 are the tril(k=1) strip mask.  Output tile i can
            # be masked in ONE op with the window msk[:, 1024-128*i :][..wl].
            msk = cpool.tile([P, T + 132], fp32)
            nc.gpsimd.memset(msk[:], 1.0)
            nc.gpsimd.affine_select(
                out=msk[:, T:], in_=msk[:, T:], compare_op=mybir.AluOpType.is_ge,
                fill=0.0, base=1, pattern=[[-1, 132]], channel_multiplier=1)
            on_sb = cpool.tile([P, P], fp32)
            nc.sync.dma_start(on_sb[:], on_d[:])
            onr = cpool.tile([P, P], f32r)
            nc.scalar.copy(onr[:], on_sb[:])
            sc_sb = cpool.tile([P, 1], fp32)
            nc.sync.dma_start(sc_sb[:], sc_d[:])
            ms_sb = cpool.tile([D, n_bh, 2 * D], fp32)
            nc.sync.dma_start(ms_sb[:], ms_d.rearrange("j d m -> d j m"))

            def body():
                for j in range(n_bh):
                    # ---- load interleaved [q|k] (t-chunk partition layout) ----
                    qk = stage.tile([P, NT, 2 * D], fp32, tag="qk")
                    nc.sync.dma_start(qk[:], qk_d[j])

                    # ---- combined transposes: kqT = [q^T ; k^T]  [128, T] ----
                    kqT = big.tile([P, T], fp32, tag="kqT")
                    for g in range(2):       # groups of 4 chunks
                        tp = trp.tile([P, 512], fp32, tag="tr")
                        for cc in range(4):
                            c = g * 4 + cc
                            nc.tensor.transpose(tp[:, 128 * cc:128 * (cc + 1)],
                                                qk[:, c, :], ident[:])
                        nc.scalar.copy(kqT[:, 512 * g:512 * (g + 1)], tp[:])

                    # ---- Ustack = [U1^T ; U2^T]  [128, 1024]  (fp32) ----
                    ust = big.tile([P, T], fp32, tag="ust")
                    for g in range(2):
                        up = usp.tile([P, 512], fp32, tag="us")
                        nc.tensor.matmul(up[:], ms_sb[:, j, :],
                                         kqT[0:D, 512 * g:512 * (g + 1)])
                        nc.scalar.copy(ust[:, 512 * g:512 * (g + 1)], up[:])

                    # duplicate kT over the consumed qT half -> [k^T ; k^T]
                    nc.sync.dma_start(kqT[0:D, :], kqT[D:P, :])

                    # ---- prefix sums C2 (cumsum of k over t, stacked) ----
                    c2 = big.tile([P, T], fp32, tag="c2")
                    nc.vector.tensor_tensor_scan(c2[:], kqT[:], kqT[:], 0.0,
                                                 mybir.AluOpType.add,
                                                 mybir.AluOpType.bypass)

                    # ---- W = Ustack * shifted(C2);  den = ones^T @ W ----
                    w_sb = big.tile([P, T], fp32, tag="w")
                    nc.gpsimd.tensor_mul(w_sb[:, 0:T - 1], ust[:, 0:T - 1],
                                         c2[:, 1:T])
                    nc.gpsimd.tensor_mul(w_sb[:, T - 1:T], ust[:, T - 1:T],
                                         c2[:, T - 1:T])

                    # hi/lo split of W keeps the den column sums at fp32
                    # accuracy while running the reduction matmul in f32r
                    # (2-pass PSUM accumulate, 1 PE cycle/row instead of 4):
                    # den = ones @ tf32(W) + ones @ (W - tf32(W))
                    wh = big.tile([P, T], f32r, tag="wh")
                    nc.scalar.copy(wh[:], w_sb[:])
                    wlo = big.tile([P, T], f32r, tag="wlo")
                    nc.vector.tensor_sub(wlo[:], w_sb[:], wh[:])

                    rden = big.tile([P, T], fp32, tag="rden")
                    for g in range(2):
                        dp = denp.tile([P, 512], fp32, tag="den")
                        nc.tensor.matmul(dp[:], onr[:],
                                         wh[:, 512 * g:512 * (g + 1)],
                                         start=True, stop=False)
                        nc.tensor.matmul(dp[:], onr[:],
                                         wlo[:, 512 * g:512 * (g + 1)],
                                         start=False, stop=True)
                        nc.vector.reciprocal(rden[:, 512 * g:512 * (g + 1)],
                                             dp[:])

                    # ---- f32r (TF32) copies for the fast output matmul ----
                    # (the fp32 kqT/ust stay exact for the den path; the
                    # verifier requires f32r matmul operands to be written
                    # pre-rounded by their producer)
                    kr = big.tile([P, T], f32r, tag="kr")
                    nc.scalar.copy(kr[:], kqT[:])

                    # ---- V = Ustack * rden * [1; -lam]  (one fused DVE op) ----
                    v_sb = big.tile([P, T], f32r, tag="v")
                    nc.vector.scalar_tensor_tensor(
                        v_sb[:], rden[:], sc_sb[:], ust[:],
                        mybir.AluOpType.mult, mybir.AluOpType.mult)

                    # ---- output tiles (f32r matmul, bf16 store) ----
                    for i in range(NT):
                        wl = _live_width(i)
                        ops = owp.tile([P, 1024], fp32, tag="ow")
                        # f32r matmul: >=256 cols for full rate, even width
                        n0 = min(wl, 512)
                        p0 = 256 if n0 < 256 else (n0 + 1) // 2 * 2
                        nc.tensor.matmul(
                            ops[:, 0:p0],
                            v_sb[:, 128 * i:128 * (i + 1)],
                            kr[:, 0:p0])
                        if wl > 512:
                            n1 = wl - 512
                            p1 = 256 if 64 < n1 < 256 else (n1 + 1) // 2 * 2
                            nc.tensor.matmul(
                                ops[:, 512:512 + p1],
                                v_sb[:, 128 * i:128 * (i + 1)],
                                kr[:, 512:512 + p1])

                        osb = outp.tile([P, 1032], bf16, tag="osb")
                        # convert+mask PSUM -> bf16 SBUF.  Pool can't touch
                        # PSUM, so balance DVE and Act: tiles in ACT_BODY get
                        # an Act body copy + DVE strip mul; the rest are one
                        # DVE mul against the windowed combined mask.
                        mw = wl - 128 * i
                        if i in (2, 4, 6, 7):
                            nc.vector.tensor_mul(osb[:, 128 * i:wl],
                                                 ops[:, 128 * i:wl],
                                                 msk[:, T:T + mw])
                            nc.scalar.copy(osb[:, 0:128 * i],
                                           ops[:, 0:128 * i])
                        else:
                            nc.vector.tensor_mul(
                                osb[:, 0:wl], ops[:, 0:wl],
                                msk[:, T - 128 * i:T - 128 * i + wl])
                        nc.sync.dma_start(
                            out_d[j, 128 * i:128 * (i + 1), 0:wl],
                            osb[:, 0:wl])

            if repeat == 1:
                body()
            else:
                with tc.For_i(0, repeat) as _:
                    body()

    nc.compile()
    _BUILD_CACHE[key] = nc
    return nc


def _host_prep(W1_q, W1_k, W2_q, W2_k, lambda_q1, lambda_k1, lambda_q2,
               lambda_k2):
    lam1 = np.exp(np.asarray(lambda_q1, np.float64).dot(
        np.asarray(lambda_k1, np.float64)))
    lam2 = np.exp(np.asarray(lambda_q2, np.float64).dot(
        np.asarray(lambda_k2, np.float64)))
    lam = np.float32(np.float32(lam1) - np.float32(lam2) + np.float32(LAMBDA_INIT))
    M1 = np.einsum("hde,hfe->hdf", W1_q.astype(np.float32),
                   W1_k.astype(np.float32)).astype(np.float32)
    M2 = np.einsum("hde,hfe->hdf", W2_q.astype(np.float32),
                   W2_k.astype(np.float32)).astype(np.float32)
    m_stack = np.concatenate([M1, M2], axis=2)          # [H, 64, 128]
    ones = np.zeros((128, 128), np.float32)
    ones[0:64, 0:64] = 1.0
    ones[64:128, 64:128] = 1.0
    sc = np.ones((128, 1), np.float32)
    sc[64:128] = -lam
    return m_stack, ones, sc


def _make_in_maps(query_states, key_states, W1_q, W1_k, W2_q, W2_k,
                  lambda_q1, lambda_k1, lambda_q2, lambda_k2):
    q = np.asarray(query_states, np.float32).reshape(BH, T, D)
    k = np.asarray(key_states, np.float32).reshape(BH, T, D)
    # interleave to [bh, p, chunk, {q,k}, d]: one contiguous 4KB DMA run
    # per partition, and each chunk's [q|k] 128-col block is contiguous
    # for the combined PE transpose.
    qs = q.reshape(BH, NT, 128, 1, D).swapaxes(1, 2)
    ks = k.reshape(BH, NT, 128, 1, D).swapaxes(1, 2)
    qk = np.ascontiguousarray(np.concatenate([qs, ks], axis=3)).reshape(
        BH, 128, NT, 2 * D)
    m_stack, ones, sc = _host_prep(W1_q, W1_k, W2_q, W2_k,
                                   lambda_q1, lambda_k1, lambda_q2, lambda_k2)
    in_maps = []
    for c in range(NCORES):
        sl = slice(c * JPC, (c + 1) * JPC)
        hs = [bh % H for bh in range(c * JPC, (c + 1) * JPC)]
        in_maps.append({
            "qk": np.ascontiguousarray(qk[sl]),
            "ms": np.ascontiguousarray(m_stack[hs]),
            "on": ones,
            "sc": sc,
        })
    return in_maps


def kernel(query_states, key_states, W1_q, W1_k, W2_q, W2_k,
           lambda_q1, lambda_k1, lambda_q2, lambda_k2):
    from concourse.bass_utils import run_bass_kernel_spmd

    in_maps = _make_in_maps(query_states, key_states, W1_q, W1_k, W2_q, W2_k,
                            lambda_q1, lambda_k1, lambda_q2, lambda_k2)
    nc = _build_module()
    res = run_bass_kernel_spmd(nc, in_maps, core_ids=list(range(NCORES)),
                               trace=False)
    out = np.empty((BH, T, T), np.float32)
    for c in range(NCORES):
        out[c * JPC:(c + 1) * JPC] = np.asarray(
            res.results[c]["out"]).astype(np.float32)
    return out.reshape(B, H, T, T)


# revision 47
# speedup vs baseline: 3.6519x; 1.0678x over previous
"""DiffLinearAttentionWeights Trainium2 kernel.

Math (per b, h):
    aw_i = (q @ Wq_i) @ (k @ Wk_i)^T  = q @ M_i @ k^T,   M_i = Wq_i @ Wk_i^T
    masked with tril(k=1), row-normalized; out = aw_1/den_1 - lam * aw_2/den_2.

Key factorizations used on-device:
  * U_i = q @ M_i  (so aw_i = U_i @ k^T, contraction over D=64).
  * den_i[t] = sum_{s<=t+1} aw_i[t,s] = U_i[t] . P[t+1], where P is the
    inclusive prefix-sum of k rows -> one DVE scan, no O(T^2) work.
  * Normalization + lambda-combination folded into the matmul: scale the
    stacked U^T columns by [1/den_1 ; -lam/den_2] and do ONE K=128 matmul
        out_tile = [U1s; U2s]^T @ [k^T; k^T]
    per 128-row output tile (only the columns under the causal profile).
  * Output rows above the causal profile are never written; the PJRT output
    buffers are donated zero-filled arrays, so skipped regions stay zero.

Performance structure (per bh pair):
  * q and k are host-swizzled to [p, chunk, d] so input DMAs move 2KB
    contiguous per partition (full-rate descriptors).
  * One [128,128] PE transpose per t-chunk of the interleaved [q|k] pair
    produces qT on partitions 0:64 and kT on 64:128 in one pass.  After the
    U matmul consumes qT, a single SBUF->SBUF DMA duplicates kT over it,
    giving the stacked [k^T; k^T] operand in place.
  * The big output matmul runs in float32r (TF32, 1 PE cycle/row vs 4 for
    fp32).  The U/den/normalization path stays full fp32: the row sums
    cancel catastrophically, so den must match the fp32 reference closely.
  * Output tiles are written as bf16 (halves output DMA bytes; rel tol is
    2e-2, bf16 rounding adds <=4e-3 per element); host converts to fp32.
  * PSUM->SBUF conversions are spread across Activation and Pool engines.

Sharding: BH = 64 (b,h) pairs, 8 per core, SPMD on 8 NeuronCores.
"""

import math
import sys

sys.path.insert(0, "/opt/trn_rl_repo")

import numpy as np

B, H, T, D = 4, 16, 1024, 64
NCORES = 8
BH = B * H
JPC = BH // NCORES          # bh pairs per core
NT = T // 128               # t-chunks of 128 rows
DEPTH = 12
LAMBDA_INIT = 0.8 - 0.6 * math.exp(-0.3 * DEPTH)

# live width of output row-block i: causal tril(k=1) keeps cols 0..128*(i+1)+1
def _live_width(i):
    return min(128 * (i + 1) + 1, T)


# packed-causal output layout: partition r holds [tile0 row r | tile1 row r
# | ...] so the per-bh store is ONE full-rate DMA of 128 contiguous rows.
_OFF = [0]
for _i in range(NT):
    _OFF.append(_OFF[-1] + _live_width(_i))
CW = _OFF[NT]               # 4615 packed causal cols per partition


_BUILD_CACHE = {}


def _build_module(n_bh=JPC, repeat=1, use_f32r=True, out_bf16=True):
    """Trace + compile the per-core Bass module (cached).

    repeat > 1 wraps the whole per-core body in a hardware loop running the
    identical (idempotent) computation `repeat` times -- used only by the
    timing harness to amortize dispatch overhead.
    """
    key = (n_bh, repeat, use_f32r, out_bf16)
    if key in _BUILD_CACHE:
        return _BUILD_CACHE[key]

    import concourse.bass as bass
    import concourse.mybir as mybir
    import concourse.bacc as bacc
    import concourse.tile as tile
    from concourse import masks

    fp32 = mybir.dt.float32
    f32r = mybir.dt.float32r if use_f32r else mybir.dt.float32
    bf16 = mybir.dt.bfloat16 if out_bf16 else mybir.dt.float32
    P = 128

    nc = bacc.Bacc("TRN2", target_bir_lowering=False, debug=False,
                   enable_asserts=False)

    # host-interleaved: element [j, p, c, 0, d] = q[j, c*128+p, d],
    #                   element [j, p, c, 1, d] = k[j, c*128+p, d]
    qk_d = nc.dram_tensor("qk", [n_bh, P, NT, 2 * D], fp32,
                          kind="ExternalInput")
    # m_stack[j] = [M1 | M2]  (64 x 128)
    ms_d = nc.dram_tensor("ms", [n_bh, D, 2 * D], fp32, kind="ExternalInput")
    # block-diagonal ones (rows 0:64 x cols 0:64 and 64:128 x 64:128) --
    # exact in TF32, so the den matmul can run as 2-pass f32r
    on_d = nc.dram_tensor("on", [P, P], fp32, kind="ExternalInput")
    # per-partition scale column: rows 0:64 = 1.0, rows 64:128 = -lambda
    sc_d = nc.dram_tensor("sc", [P, 1], fp32, kind="ExternalInput")
    # packed causal layout [j, r, CW]; host unpacks to [T, T]
    out_d = nc.dram_tensor("out", [n_bh, P, CW], bf16, kind="ExternalOutput")

    with tile.TileContext(nc) as tc:
        with tc.tile_pool(name="const", bufs=1) as cpool, \
             tc.tile_pool(name="stage", bufs=3) as stage, \
             tc.tile_pool(name="big", bufs=3) as big, \
             tc.tile_pool(name="outp", bufs=4) as outp, \
             tc.tile_pool(name="trp", bufs=2, space=bass.MemorySpace.PSUM) as trp, \
             tc.tile_pool(name="usp", bufs=1, space=bass.MemorySpace.PSUM) as usp, \
             tc.tile_pool(name="denp", bufs=1, space=bass.MemorySpace.PSUM) as denp, \
             tc.tile_pool(name="owp", bufs=2, space=bass.MemorySpace.PSUM) as owp:

            # ---- constants ----
            ident = cpool.tile([P, P], fp32)
            masks.make_identity(nc, ident[:])
            # combined causal mask: cols 0..1023 are all-ones (full-keep body),
            # cols 1024..1155 are the tril(k=1) strip mask.  Output tile i can
            # be masked in ONE op with the window msk[:, 1024-128*i :][..wl].
            msk = cpool.tile([P, T + 132], fp32)
            nc.gpsimd.memset(msk[:], 1.0)
            nc.gpsimd.affine_select(
                out=msk[:, T:], in_=msk[:, T:], compare_op=mybir.AluOpType.is_ge,
                fill=0.0, base=1, pattern=[[-1, 132]], channel_multiplier=1)
            on_sb = cpool.tile([P, P], fp32)
            nc.sync.dma_start(on_sb[:], on_d[:])
            sc_sb = cpool.tile([P, 1], fp32)
            nc.sync.dma_start(sc_sb[:], sc_d[:])
            ms_sb = cpool.tile([D, n_bh, 2 * D], fp32)
            nc.sync.dma_start(ms_sb[:], ms_d.rearrange("j d m -> d j m"))

            def body():
                for j in range(n_bh):
                    # ---- load interleaved [q|k] (t-chunk partition layout) ----
                    qk = stage.tile([P, NT, 2 * D], fp32, tag="qk")
                    nc.sync.dma_start(qk[:], qk_d[j])

                    # ---- combined transposes: kqT = [q^T ; k^T]  [128, T] ----
                    kqT = big.tile([P, T], fp32, tag="kqT")
                    for g in range(2):       # groups of 4 chunks
                        tp = trp.tile([P, 512], fp32, tag="tr")
                        for cc in range(4):
                            c = g * 4 + cc
                            nc.tensor.transpose(tp[:, 128 * cc:128 * (cc + 1)],
                                                qk[:, c, :], ident[:])
                        nc.scalar.copy(kqT[:, 512 * g:512 * (g + 1)], tp[:])

                    # ---- Ustack = [U1^T ; U2^T]  [128, 1024]  (fp32) ----
                    ust = big.tile([P, T], fp32, tag="ust")
                    for g in range(2):
                        up = usp.tile([P, 512], fp32, tag="us")
                        nc.tensor.matmul(up[:], ms_sb[:, j, :],
                                         kqT[0:D, 512 * g:512 * (g + 1)])
                        nc.scalar.copy(ust[:, 512 * g:512 * (g + 1)], up[:])

                    # duplicate kT over the consumed qT half -> [k^T ; k^T]
                    nc.sync.dma_start(kqT[0:D, :], kqT[D:P, :])

                    # ---- prefix sums C2 (cumsum of k over t, stacked) ----
                    c2 = big.tile([P, T], fp32, tag="c2")
                    nc.vector.tensor_tensor_scan(c2[:], kqT[:], kqT[:], 0.0,
                                                 mybir.AluOpType.add,
                                                 mybir.AluOpType.bypass)

                    # ---- W = Ustack * shifted(C2);  den = ones^T @ W ----
                    w_sb = big.tile([P, T], fp32, tag="w")
                    nc.gpsimd.tensor_mul(w_sb[:, 0:T - 1], ust[:, 0:T - 1],
                                         c2[:, 1:T])
                    nc.gpsimd.tensor_mul(w_sb[:, T - 1:T], ust[:, T - 1:T],
                                         c2[:, T - 1:T])

                    # den column sums: fp32 matmul against the block-diag
                    # ones (den needs full fp32 accuracy -- the row sums
                    # cancel catastrophically)
                    rden = big.tile([P, T], fp32, tag="rden")
                    for g in range(2):
                        dp = denp.tile([P, 512], fp32, tag="den")
                        nc.tensor.matmul(dp[:], on_sb[:],
                                         w_sb[:, 512 * g:512 * (g + 1)])
                        nc.vector.reciprocal(rden[:, 512 * g:512 * (g + 1)],
                                             dp[:])

                    # ---- f32r (TF32) copies for the fast output matmul ----
                    # (the fp32 kqT/ust stay exact for the den path; the
                    # verifier requires f32r matmul operands to be written
                    # pre-rounded by their producer)
                    kr = big.tile([P, T], f32r, tag="kr")
                    nc.scalar.copy(kr[:], kqT[:])

                    # ---- V = Ustack * rden * [1; -lam]  (one fused DVE op) ----
                    v_sb = big.tile([P, T], f32r, tag="v")
                    nc.vector.scalar_tensor_tensor(
                        v_sb[:], rden[:], sc_sb[:], ust[:],
                        mybir.AluOpType.mult, mybir.AluOpType.mult)

                    # ---- output tiles (f32r matmul, bf16 store) ----
                    # converted tiles accumulate into one packed SBUF buffer
                    # so the store is a single 128 x 9.2KB full-rate DMA
                    osb = outp.tile([P, CW], bf16, tag="osb")
                    for i in range(NT):
                        wl = _live_width(i)
                        ops = owp.tile([P, 1024], fp32, tag="ow")
                        # f32r matmul: >=256 cols for full rate, even width
                        n0 = min(wl, 512)
                        p0 = 256 if n0 < 256 else (n0 + 1) // 2 * 2
                        nc.tensor.matmul(
                            ops[:, 0:p0],
                            v_sb[:, 128 * i:128 * (i + 1)],
                            kr[:, 0:p0])
                        if wl > 512:
                            n1 = wl - 512
                            p1 = 256 if 64 < n1 < 256 else (n1 + 1) // 2 * 2
                            nc.tensor.matmul(
                                ops[:, 512:512 + p1],
                                v_sb[:, 128 * i:128 * (i + 1)],
                                kr[:, 512:512 + p1])

                        # convert+mask PSUM -> bf16 SBUF.  Pool can't touch
                        # PSUM, so balance DVE and Act: tiles in ACT_BODY get
                        # an Act body copy + DVE strip mul; the rest are one
                        # DVE mul against the windowed combined mask.
                        o0 = _OFF[i]
                        mw = wl - 128 * i
                        if i in (2, 4, 6, 7):
                            nc.vector.tensor_mul(osb[:, o0 + 128 * i:o0 + wl],
                                                 ops[:, 128 * i:wl],
                                                 msk[:, T:T + mw])
                            nc.scalar.copy(osb[:, o0:o0 + 128 * i],
                                           ops[:, 0:128 * i])
                        else:
                            nc.vector.tensor_mul(
                                osb[:, o0:o0 + wl], ops[:, 0:wl],
                                msk[:, T - 128 * i:T - 128 * i + wl])
                    nc.sync.dma_start(out_d[j], osb[:])

            if repeat == 1:
                body()
            else:
                with tc.For_i(0, repeat) as _:
                    body()

    nc.compile()
    _BUILD_CACHE[key] = nc
    return nc


def _host_prep(W1_q, W1_k, W2_q, W2_k, lambda_q1, lambda_k1, lambda_q2,
               lambda_k2):
    lam1 = np.exp(np.asarray(lambda_q1, np.float64).dot(
        np.asarray(lambda_k1, np.float64)))
    lam2 = np.exp(np.asarray(lambda_q2, np.float64).dot(
        np.asarray(lambda_k2, np.float64)))
    lam = np.float32(np.float32(lam1) - np.float32(lam2) + np.float32(LAMBDA_INIT))
    M1 = np.einsum("hde,hfe->hdf", W1_q.astype(np.float32),
                   W1_k.astype(np.float32)).astype(np.float32)
    M2 = np.einsum("hde,hfe->hdf", W2_q.astype(np.float32),
                   W2_k.astype(np.float32)).astype(np.float32)
    m_stack = np.concatenate([M1, M2], axis=2)          # [H, 64, 128]
    ones = np.zeros((128, 128), np.float32)
    ones[0:64, 0:64] = 1.0
    ones[64:128, 64:128] = 1.0
    sc = np.ones((128, 1), np.float32)
    sc[64:128] = -lam
    return m_stack, ones, sc


def _make_in_maps(query_states, key_states, W1_q, W1_k, W2_q, W2_k,
                  lambda_q1, lambda_k1, lambda_q2, lambda_k2):
    q = np.asarray(query_states, np.float32).reshape(BH, T, D)
    k = np.asarray(key_states, np.float32).reshape(BH, T, D)
    # interleave to [bh, p, chunk, {q,k}, d]: one contiguous 4KB DMA run
    # per partition, and each chunk's [q|k] 128-col block is contiguous
    # for the combined PE transpose.
    qs = q.reshape(BH, NT, 128, 1, D).swapaxes(1, 2)
    ks = k.reshape(BH, NT, 128, 1, D).swapaxes(1, 2)
    qk = np.ascontiguousarray(np.concatenate([qs, ks], axis=3)).reshape(
        BH, 128, NT, 2 * D)
    m_stack, ones, sc = _host_prep(W1_q, W1_k, W2_q, W2_k,
                                   lambda_q1, lambda_k1, lambda_q2, lambda_k2)
    in_maps = []
    for c in range(NCORES):
        sl = slice(c * JPC, (c + 1) * JPC)
        hs = [bh % H for bh in range(c * JPC, (c + 1) * JPC)]
        in_maps.append({
            "qk": np.ascontiguousarray(qk[sl]),
            "ms": np.ascontiguousarray(m_stack[hs]),
            "on": ones,
            "sc": sc,
        })
    return in_maps


def kernel(query_states, key_states, W1_q, W1_k, W2_q, W2_k,
           lambda_q1, lambda_k1, lambda_q2, lambda_k2):
    from concourse.bass_utils import run_bass_kernel_spmd

    in_maps = _make_in_maps(query_states, key_states, W1_q, W1_k, W2_q, W2_k,
                            lambda_q1, lambda_k1, lambda_q2, lambda_k2)
    nc = _build_module()
    res = run_bass_kernel_spmd(nc, in_maps, core_ids=list(range(NCORES)),
                               trace=False)
    out = np.zeros((BH, T, T), np.float32)
    for c in range(NCORES):
        packed = np.asarray(res.results[c]["out"])   # [JPC, 128, CW] bf16
        sl = slice(c * JPC, (c + 1) * JPC)
        for i in range(NT):
            wl = _live_width(i)
            out[sl, 128 * i:128 * (i + 1), 0:wl] = packed[
                :, :, _OFF[i]:_OFF[i] + wl].astype(np.float32)
    return out.reshape(B, H, T, T)
